# revision 1
# baseline (speedup 1.0000x reference)
"""CLCE loss kernel for Trainium2 (8 NeuronCores, SPMD).

Loss = 0.5 * cl + 0.5 * ce where
  cl_i = logsumexp(loss_temp_i) - slot0_i   over a [N, 2N-1] packed row
  ce   = cross-entropy of y_pred vs y_true.

Decomposition (exact, validated in f64 against the reference formula):
  cl_i = log(exp(slot0_i) + (T_i - P_i) + (2N-2 - num_neg_i)) - slot0_i
where
  T_i  = sum_j exp((xn_i . xn_j + 1) * 0.25)      <- the O(N^2 D) part, on device
  P_i  = sum_{j: y_j = y_i} exp(sim_ij)           <- O(N * class_size), on host
  slot0_i = sim_{i, first same-class j != i}      <- O(N), on host
  R_i  = sum_j exp(y_pred_ij)                     <- on device
  ce_i = log(R_i) - y_pred[i, y_i]

Device sharding: core c computes rows [512c, 512(c+1)) of the similarity
matrix as an fp8e4m3 DoubleRow matmul (2 MACs/cell/cycle; embeddings are
pre-scaled by S8 on the host so quantization error stays ~1e-4 relative on
each sim entry, which averages to ~1e-5 on the final scalar loss), with the
exp+row-sum fused into Scalar-engine activations (accum_out).  The
correction terms P_i/slot0_i are computed on the host in full precision
from the same normalized embeddings, so the handful of same-class entries
inside T_i cancel to fp8-noise level.
"""

import os
from contextlib import ExitStack

import numpy as np

import concourse.bass as bass
import concourse.tile as tile
from concourse import bacc, mybir
from concourse.bass_utils import run_bass_kernel_spmd

N, D, C = 4096, 1024, 512
TAU = 0.5
LAMBD = 0.5
NCORES = 8
BLK = N // NCORES          # 512 rows per core
P = 128                    # partitions
KT = D // 256              # 4 DoubleRow contraction super-tiles (256 each)
MT = BLK // P              # 4 output row tiles per core
W = 1024                   # column-chunk width (2 psum banks)
HC = N // W                # 4 column chunks
NS = W // 512              # matmuls per chunk k-step
S8 = 16.0                  # fp8 pre-scale for the embeddings

_F32 = mybir.dt.float32
_FP8 = mybir.dt.float8e4
_EXP = mybir.ActivationFunctionType.Exp
_DR = mybir.MatmulPerfMode.DoubleRow


def _build_kernel(tc, xt, wt, yp, out):
    """Emit the per-core Tile kernel.

    xt:  [KT*P, 2*N]   fp8  row kk*128+p, col i*N+n = S8*xn[n, kk*256+128i+p]
    wt:  [KT*P, 2*BLK] fp8  this core's column block, same packing
    yp:  [P, MT*C]     f32  this core's y_pred block, partition-major packed
    out: [P, MT*HC+MT] f32  T chunk-sums then R row-sums
    """
    nc = tc.nc
    with ExitStack() as ctx:
        pers = ctx.enter_context(tc.tile_pool(name="pers", bufs=1))
        epool = ctx.enter_context(tc.tile_pool(name="epool", bufs=2))
        psum = ctx.enter_context(
            tc.tile_pool(name="psum", bufs=4, space=bass.MemorySpace.PSUM)
        )

        # per-(kk, h) input tiles -> exact DMA->matmul dependencies.
        # The weights and the first column chunk arrive fused in one DMA per
        # kk (WX0) to halve the issue slots pacing the pipeline start.
        WX0 = [
            pers.tile([P, 2, BLK + W], _FP8, name=f"wx0_{k}", tag=f"wx0_{k}")
            for k in range(KT)
        ]
        XT = [
            [None] + [
                pers.tile([P, 2, W], _FP8, name=f"xtt{k}_{h}", tag=f"xtt{k}_{h}")
                for h in range(1, HC)
            ]
            for k in range(KT)
        ]
        WT = [WX0[k][:, :, 0:BLK] for k in range(KT)]
        for k in range(KT):
            XT[k][0] = WX0[k][:, :, BLK:BLK + W]
        YPB = pers.tile([P, MT * C], _F32)     # 8 KiB/partition
        # out layout: [Tparts (MT*HC) | Rparts (MT)]
        OUTSB = pers.tile([P, MT * HC + MT], _F32)
        bias_s = pers.tile([P, 1], _F32)       # 0.5*TAU for the sim affine
        bias_z = pers.tile([P, 1], _F32)       # 0.0 for plain exp
        warm = pers.tile([P, 1], _F32)

        ZW = pers.tile([P, 512], mybir.dt.bfloat16)  # zeros, PE warm-up operand

        nc.gpsimd.memset(ZW[:], 0.0)
        nc.gpsimd.memset(bias_s[:], 0.5 * TAU)
        nc.gpsimd.memset(bias_z[:], 0.0)
        # warm the exp table (ACT_TABLE_LOAD ~2.7us) before any data lands
        nc.scalar.activation(warm[:], bias_z[:], _EXP, bias=bias_z[:], scale=1.0)

        # PE warm-up: dummy matmuls spanning the input-DMA latency (~7us)
        # flip the HAM clock gate to 8/8 so the real stream starts at 2.4GHz
        wps = psum.tile([P, W], _F32, tag="ps")
        for _ in range(12):
            nc.tensor.matmul(wps[:, 0:512], ZW[:, 0:P], ZW[:], start=True, stop=True)

        # --- input DMAs.  Sync HWDGE carries the matmul operands in exactly
        # the order the PE consumes them: (WT kk, XT[kk][0]) pairs pace the
        # first chunk, then the later column chunks.  y_pred rides the
        # scalar HWDGE queue so it neither delays the sync stream nor the
        # CE activations. ---
        nc.scalar.dma_start(YPB[:], yp[:])
        xt3 = xt.rearrange("r (i n) -> r i n", i=2)
        wt3 = wt.rearrange("r (i n) -> r i n", i=2)
        for k in range(KT):
            nc.sync.dma_start(WX0[k][:], wt3[k * P:(k + 1) * P, :, :])
        for h in range(1, HC):
            for k in range(KT):
                nc.sync.dma_start(
                    XT[k][h][:],
                    xt3[k * P:(k + 1) * P, :, h * W:(h + 1) * W],
                )

        # --- CE: R[p, t] = sum_c exp(y_pred) ---
        for t in range(MT):
            et = epool.tile([P, W], _F32)
            nc.scalar.activation(
                et[:, 0:C], YPB[:, t * C:(t + 1) * C], _EXP,
                bias=bias_z[:], scale=1.0,
                accum_out=OUTSB[:, MT * HC + t:MT * HC + t + 1],
            )

        # --- main: sim block matmul + fused exp/row-sum ---
        # dot_scaled = S8^2 * xn_i . xn_j ; sim = (dot + 1) * 0.5 * TAU
        # -> exp(scale * dot_scaled + bias), scale = 0.5*TAU/S8^2, bias = 0.25
        act_scale = 0.5 * TAU / (S8 * S8)

        # first column chunk: k-outer over m=0..2 so the PE does three
        # m-tiles' work per arriving (WT k, XT k) pair -- stays dense behind
        # the DMA stream instead of stalling per k (which would re-throttle
        # the clock gate).  m=3 runs as a pipelined chunk afterward so its
        # matmuls cover the m=0..2 exp/row-sum drain and h=1 starts with a
        # free psum slot.
        ps_h0 = [
            psum.tile([P, W], _F32, tag="ps", name=f"psh0_{m}")
            for m in range(MT - 1)
        ]
        for k in range(KT):
            for m in range(MT - 1):
                for ns in range(NS):
                    nc.tensor.matmul(
                        ps_h0[m][:, ns * 512:(ns + 1) * 512],
                        WT[k][:, :, m * P:(m + 1) * P],
                        XT[k][0][:, :, ns * 512:(ns + 1) * 512],
                        start=(k == 0),
                        stop=(k == KT - 1),
                        perf_mode=_DR,
                    )
        for m in range(MT - 1):
            et = epool.tile([P, W], _F32)
            nc.scalar.activation(
                et[:], ps_h0[m][:], _EXP,
                bias=bias_s[:], scale=act_scale,
                accum_out=OUTSB[:, m * HC:m * HC + 1],
            )

        # remaining chunks: m-outer with psum-pool ping-pong (zero steady
        # state PE stalls; exp+row-sum runs concurrently on ScalarE)
        for h, m in [(0, MT - 1)] + [
            (h, m) for h in range(1, HC) for m in range(MT)
        ]:
            if True:
                ps = psum.tile([P, W], _F32, tag="ps")
                for k in range(KT):
                    for ns in range(NS):
                        nc.tensor.matmul(
                            ps[:, ns * 512:(ns + 1) * 512],
                            WT[k][:, :, m * P:(m + 1) * P],
                            XT[k][h][:, :, ns * 512:(ns + 1) * 512],
                            start=(k == 0),
                            stop=(k == KT - 1),
                            perf_mode=_DR,
                        )
                et = epool.tile([P, W], _F32)
                nc.scalar.activation(
                    et[:], ps[:], _EXP,
                    bias=bias_s[:], scale=act_scale,
                    accum_out=OUTSB[:, m * HC + h:m * HC + h + 1],
                )

        nc.scalar.dma_start(out[:], OUTSB[:])


_NC_CACHE = None


def _get_nc():
    global _NC_CACHE
    if _NC_CACHE is None:
        nc = bacc.Bacc(
            "TRN2", target_bir_lowering=False, debug=False,
            enable_asserts=False, num_devices=NCORES,
        )
        xt_d = nc.dram_tensor("xt", [KT * P, 2 * N], _FP8, kind="ExternalInput")
        wt_d = nc.dram_tensor(
            "wt", [KT * P, 2 * (BLK + W)], _FP8, kind="ExternalInput"
        )
        yp_d = nc.dram_tensor("yp", [P, MT * C], _F32, kind="ExternalInput")
        out_d = nc.dram_tensor(
            "out", [P, MT * HC + MT], _F32, kind="ExternalOutput"
        )
        with tile.TileContext(nc) as tc:
            _build_kernel(tc, xt_d.ap(), wt_d.ap(), yp_d.ap(), out_d.ap())
        nc.compile()
        _NC_CACHE = nc
    return _NC_CACHE


def _pack_fp8(zT, cols):
    """[D, ncols] f32 -> [KT*P, 2*ncols] fp8 with the DoubleRow pairing
    row kk*128+p, col i*ncols+n  <->  contraction index kk*256 + 128i + p."""
    fp8np = mybir.dt.np(_FP8)
    q = zT.reshape(KT, 2, P, cols).transpose(0, 2, 1, 3).reshape(KT * P, 2 * cols)
    return np.ascontiguousarray(q.astype(fp8np))


def _run_device(xnT, y_pred, trace=False):
    """Run the SPMD kernel; returns (T[N], R[N]) f64 and the raw results."""
    zT = (xnT * S8).astype(np.float32)  # [D, N], pre-scaled
    xt8 = _pack_fp8(zT, N)
    in_maps = []
    for c in range(NCORES):
        blk = slice(c * BLK, (c + 1) * BLK)
        ypb = (
            np.ascontiguousarray(y_pred[blk])
            .reshape(MT, P, C).transpose(1, 0, 2).reshape(P, MT * C)
        )
        wx0 = np.concatenate([zT[:, blk], zT[:, 0:W]], axis=1)
        in_maps.append({
            "xt": xt8,
            "wt": _pack_fp8(np.ascontiguousarray(wx0), BLK + W),
            "yp": np.ascontiguousarray(ypb),
        })
    res = run_bass_kernel_spmd(
        _get_nc(), in_maps, core_ids=list(range(NCORES)), trace=trace,
    )
    T = np.empty(N, np.float64)
    R = np.empty(N, np.float64)
    for c, r in enumerate(res.results):
        o = r["out"].astype(np.float64)  # [128, MT*HC + MT]
        for m in range(MT):
            rows = slice(c * BLK + m * P, c * BLK + (m + 1) * P)
            T[rows] = o[:, m * HC:(m + 1) * HC].sum(axis=1)
            R[rows] = o[:, MT * HC + m]
    return T, R, res


def kernel(layer_embeds, y_true, y_pred):
    x = np.asarray(layer_embeds, dtype=np.float32)
    yt = np.asarray(y_true).astype(np.int64)
    yp = np.asarray(y_pred, dtype=np.float32)

    # normalize rows (torch-style eps clip)
    norms = np.maximum(
        np.sqrt((x.astype(np.float64) ** 2).sum(1, keepdims=True)), 1e-8
    )
    xn = (x / norms).astype(np.float32)
    xnT = np.ascontiguousarray(xn.T)  # [D, N]

    trace = bool(int(os.environ.get("CLCE_TRACE", "0")))
    T, R, res = _run_device(xnT, yp, trace=trace)
    if trace:
        kernel.last_results = res

    # --- host-side small terms (O(N * class_size)) ---
    # P_ must match what the device summed for the same-class entries, i.e.
    # the fp8-quantized sim values, so quantize the same way here.
    fp8np = mybir.dt.np(_FP8)
    xq = (xn * S8).astype(fp8np).astype(np.float64) / S8  # device-visible xn
    counts = np.bincount(yt, minlength=C)
    P_ = np.zeros(N, np.float64)
    slot0 = np.zeros(N, np.float64)
    for cval in np.unique(yt):
        idx = np.where(yt == cval)[0]
        subq = xq[idx]
        sq = (subq @ subq.T + 1.0) * (0.5 * TAU)   # device-matching sim
        P_[idx] = np.exp(sq).sum(1)
        if len(idx) >= 2:
            # slot0 feeds the final formula directly -> use full precision
            sub = xn[idx].astype(np.float64)
            s = (sub @ sub.T + 1.0) * (0.5 * TAU)
            firstpos = np.where(np.arange(len(idx)) == 0, 1, 0)
            slot0[idx] = s[np.arange(len(idx)), firstpos]

    num_neg = N - counts[yt]
    S = T - P_
    Z = (2 * N - 2 - num_neg).astype(np.float64)
    cl = (np.log(np.exp(slot0) + S + Z) - slot0).mean()
    ce = (
        np.log(R) - yp[np.arange(N), yt].astype(np.float64)
    ).mean()
    loss = LAMBD * cl + (1.0 - LAMBD) * ce
    return np.asarray(loss, dtype=np.float32)



# revision 4
# speedup vs baseline: 1.0288x; 1.0288x over previous
"""CLCE loss kernel for Trainium2 (8 NeuronCores, SPMD) — symmetric-triangle.

Loss = 0.5 * cl + 0.5 * ce where
  cl_i = log(exp(slot0_i) + (T_i - P_i) + (2N-2 - num_neg_i)) - slot0_i
  T_i  = sum_j exp((xn_i . xn_j + 1) * 0.25)      <- O(N^2 D), on device
  P_i, slot0_i, ce assembled on host as in the validated baseline.

exp(sim) is symmetric, so only a triangle cover of the 8x8 grid of
512x512 blocks is computed (4.5 block-units per core instead of 8):
  core k: diag (k,k)  [row-sums only]
          (k, k+d) for d=1..3  [row-sums + column-sums]
          half of the d=4 block: rows chunk p=k%4 (m01 if k<4 else m23)
          x cols chunk p+4  [row-sums + column-sums]
Row sums come free from the Scalar-engine exp accumulator; column sums
are DVE adds of the bf16 exp tiles over the 4 m-tiles followed by a
ones-weight matmul (contraction over the 128 partitions).  Host adds the
per-block RS/CS pieces into T.  Matmul work: 72 DR-fp8 matmuls of 512
cols vs 128 in the full-row version.
"""

import os
from contextlib import ExitStack

import numpy as np

import concourse.bass as bass
import concourse.tile as tile
from concourse import bacc, mybir
from concourse.bass_utils import run_bass_kernel_spmd

N, D, C = 4096, 1024, 512
TAU = 0.5
LAMBD = 0.5
NCORES = 8
BLK = 512                  # chunk width (rows per core block)
P = 128                    # partitions
KT = D // 256              # 4 DoubleRow contraction super-tiles
MT = BLK // P              # 4 m-tiles per full block
S8 = 16.0                  # fp8 pre-scale for the embeddings
AW = 2 * BLK               # A region: c0 (W + diag X) | c1
BW = 3 * BLK + 256         # B region: c2 | c3 | c4 | wh

_F32 = mybir.dt.float32
_BF16 = mybir.dt.bfloat16
_FP8 = mybir.dt.float8e4
_EXP = mybir.ActivationFunctionType.Exp
_DR = mybir.MatmulPerfMode.DoubleRow

# OUTSB column map
RS_B1, RS_B2, RS_B4, RS_B3, RS_B0, RS_CE = 0, 4, 8, 10, 14, 18
NRS = 22
# OUTCS slot map (512 cols each)
CS_B1, CS_B2, CS_B4, CS_B3 = 0, 1, 2, 3


def _build_kernel(tc, biga, bigb, yp, out_rs, out_cs):
    """Emit the per-core Tile kernel.

    biga: [KT*P, 2*AW]  fp8  per k: [c0 | c1] chunk packs (W + first X)
    bigb: [KT*P, 2*BW]  fp8  per k: [c2 | c3 | c4 | wh]
    yp:   [P, MT*C]     f32  this core's y_pred block, partition-major
    out_rs: [P, 22]     f32  18 sim row-sum cols + 4 CE row-sums
    out_cs: [1, 2048]   f32  4 x 512 column-sum vectors (b1,b2,b4,b3)
    """
    nc = tc.nc
    act_scale = 0.5 * TAU / (S8 * S8)
    with ExitStack() as ctx:
        pers = ctx.enter_context(tc.tile_pool(name="pers", bufs=1))
        epool = ctx.enter_context(tc.tile_pool(name="epool", bufs=3))
        cepool = ctx.enter_context(tc.tile_pool(name="cepool", bufs=2))
        apool = ctx.enter_context(tc.tile_pool(name="apool", bufs=4))
        psum = ctx.enter_context(
            tc.tile_pool(name="psum", bufs=5, space=bass.MemorySpace.PSUM)
        )
        cpsum = ctx.enter_context(
            tc.tile_pool(name="cpsum", bufs=2, space=bass.MemorySpace.PSUM)
        )

        A = [pers.tile([P, 2, AW], _FP8, name=f"ba{k}", tag=f"ba{k}")
             for k in range(KT)]
        B = [pers.tile([P, 2, BW], _FP8, name=f"bb{k}", tag=f"bb{k}")
             for k in range(KT)]
        YPB = pers.tile([P, MT * C], _F32)
        OUTSB = pers.tile([P, NRS], _F32)
        OUTCS = pers.tile([1, 4 * BLK], _F32)
        bias_s = pers.tile([P, 1], _F32)
        bias_z = pers.tile([P, 1], _F32)
        warm = pers.tile([P, 1], _F32)
        ZW = pers.tile([P, 512], _BF16)
        ONES = pers.tile([P, P], _BF16)

        # gpsimd setup; ZW first (gates PE warm-up), bias_z second (gates
        # the exp-table warm on ScalarE)
        nc.gpsimd.memset(ZW[:], 0.0)
        nc.gpsimd.memset(bias_z[:], 0.0)
        nc.gpsimd.memset(bias_s[:], 0.5 * TAU)
        nc.gpsimd.memset(ONES[:], 1.0)

        # scalar: y_pred DMA rides the scalar HWDGE queue; exp-table warm
        nc.scalar.dma_start(YPB[:], yp[:])
        nc.scalar.activation(warm[:], bias_z[:], _EXP, bias=bias_z[:], scale=1.0)

        # input DMAs: A tiles (critical path, in k order) on sync; B tiles
        # split across vector/gpsimd queues
        a3 = biga.rearrange("r (i n) -> r i n", i=2)
        b3 = bigb.rearrange("r (i n) -> r i n", i=2)
        for k in range(KT):
            nc.sync.dma_start(A[k][:], a3[k * P:(k + 1) * P, :, :])
        for k in range(KT):
            nc.gpsimd.dma_start(B[k][:], b3[k * P:(k + 1) * P, :, :])

        # PE warm-up: dummy matmuls to flip the HAM clock gate while the
        # first input DMAs are in flight
        wps = psum.tile([P, BLK], _F32, tag="ps")
        for _ in range(3):
            nc.tensor.matmul(wps[:], ZW[:, 0:P], ZW[:], start=True, stop=True)

        def mm_group(ps, woff, wsrc, xoff, xsrc, m):
            """One m-tile's 4-k accumulation chain into psum tile ps."""
            for k in range(KT):
                nc.tensor.matmul(
                    ps[:],
                    wsrc[k][:, :, woff + m * P: woff + (m + 1) * P],
                    xsrc[k][:, :, xoff: xoff + BLK],
                    start=(k == 0),
                    stop=(k == KT - 1),
                    perf_mode=_DR,
                )

        def act_rs(ps, col):
            """exp + row-sum accumulate; returns the bf16 exp tile."""
            et = epool.tile([P, BLK], _BF16, tag="et")
            nc.scalar.activation(
                et[:], ps[:], _EXP, bias=bias_s[:], scale=act_scale,
                accum_out=OUTSB[:, col:col + 1],
            )
            return et

        def ones_mm(acc, slot):
            cps = cpsum.tile([P, BLK], _F32, tag="cps")
            nc.tensor.matmul(cps[:], ONES[:], acc[:], start=True, stop=True)
            nc.vector.tensor_copy(
                OUTCS[0:1, slot * BLK:(slot + 1) * BLK], cps[0:1, :]
            )

        # --- b1 (X = c1): k-outer m-inner to chase the arriving A tiles ---
        ps1 = [psum.tile([P, BLK], _F32, tag="ps", name=f"ps1_{m}")
               for m in range(MT)]
        for k in range(KT):
            for m in range(MT):
                nc.tensor.matmul(
                    ps1[m][:],
                    A[k][:, :, m * P:(m + 1) * P],
                    A[k][:, :, BLK:2 * BLK],
                    start=(k == 0),
                    stop=(k == KT - 1),
                    perf_mode=_DR,
                )
        ets1 = [act_rs(ps1[m], RS_B1 + m) for m in range(MT)]
        a01 = apool.tile([P, BLK], _BF16, tag="acc")
        nc.vector.tensor_add(a01[:], ets1[0][:], ets1[1][:])
        a23 = apool.tile([P, BLK], _BF16, tag="acc")
        nc.vector.tensor_add(a23[:], ets1[2][:], ets1[3][:])
        csa1 = apool.tile([P, BLK], _BF16, tag="acc")
        nc.vector.tensor_add(csa1[:], a01[:], a23[:])

        # --- CE: R[p, t] = sum_c exp(y_pred) (ScalarE slack window) ---
        for t in range(MT):
            etc = cepool.tile([P, C], _F32, tag="etce")
            nc.scalar.activation(
                etc[:], YPB[:, t * C:(t + 1) * C], _EXP,
                bias=bias_z[:], scale=1.0,
                accum_out=OUTSB[:, RS_CE + t:RS_CE + t + 1],
            )

        def cs_block(ets, slot, nadd):
            if nadd == 2:
                x01 = apool.tile([P, BLK], _BF16, tag="acc")
                nc.vector.tensor_add(x01[:], ets[0][:], ets[1][:])
                return x01
            x01 = apool.tile([P, BLK], _BF16, tag="acc")
            nc.vector.tensor_add(x01[:], ets[0][:], ets[1][:])
            x23 = apool.tile([P, BLK], _BF16, tag="acc")
            nc.vector.tensor_add(x23[:], ets[2][:], ets[3][:])
            xs = apool.tile([P, BLK], _BF16, tag="acc")
            nc.vector.tensor_add(xs[:], x01[:], x23[:])
            return xs

        # --- b2 (X = c2 at B offset 0), m-outer; onesMM(b1) after m2 ---
        ets2 = []
        for m in range(MT):
            ps = psum.tile([P, BLK], _F32, tag="ps")
            mm_group(ps, 0, A, 0, B, m)
            ets2.append(act_rs(ps, RS_B2 + m))
            if m == 2:
                ones_mm(csa1, CS_B1)
        csa2 = cs_block(ets2, CS_B2, 4)

        # --- b4 (half: W = wh at B offset 3*BLK+ , X = c4 at B 2*BLK) ---
        ets4 = []
        for m in range(2):
            ps = psum.tile([P, BLK], _F32, tag="ps")
            mm_group(ps, 3 * BLK, B, 2 * BLK, B, m)
            ets4.append(act_rs(ps, RS_B4 + m))
        ones_mm(csa2, CS_B2)
        csa4 = cs_block(ets4, CS_B4, 2)

        # --- b3 (X = c3 at B offset BLK); onesMM(b4) after m1 ---
        ets3 = []
        for m in range(MT):
            ps = psum.tile([P, BLK], _F32, tag="ps")
            mm_group(ps, 0, A, BLK, B, m)
            ets3.append(act_rs(ps, RS_B3 + m))
            if m == 1:
                ones_mm(csa4, CS_B4)
        csa3 = cs_block(ets3, CS_B3, 4)

        # --- b0 (diag, X = c0 at A offset 0): RS only; onesMM(b3) after m1 ---
        for m in range(MT):
            ps = psum.tile([P, BLK], _F32, tag="ps")
            mm_group(ps, 0, A, 0, A, m)
            act_rs(ps, RS_B0 + m)
            if m == 1:
                ones_mm(csa3, CS_B3)
                nc.gpsimd.dma_start(out_cs[:], OUTCS[:])

        nc.scalar.dma_start(out_rs[:], OUTSB[:])


_NC_CACHE = None


def _get_nc():
    global _NC_CACHE
    if _NC_CACHE is None:
        nc = bacc.Bacc(
            "TRN2", target_bir_lowering=False, debug=False,
            enable_asserts=False, num_devices=NCORES,
        )
        biga_d = nc.dram_tensor("biga", [KT * P, 2 * AW], _FP8, kind="ExternalInput")
        bigb_d = nc.dram_tensor("bigb", [KT * P, 2 * BW], _FP8, kind="ExternalInput")
        yp_d = nc.dram_tensor("yp", [P, MT * C], _F32, kind="ExternalInput")
        out_rs_d = nc.dram_tensor("out_rs", [P, NRS], _F32, kind="ExternalOutput")
        out_cs_d = nc.dram_tensor("out_cs", [1, 4 * BLK], _F32, kind="ExternalOutput")
        with tile.TileContext(nc) as tc:
            _build_kernel(
                tc, biga_d.ap(), bigb_d.ap(), yp_d.ap(),
                out_rs_d.ap(), out_cs_d.ap(),
            )
        nc.compile()
        _NC_CACHE = nc
    return _NC_CACHE


def _pack_cols(zq, cols_list):
    """[D, *] fp8 col-chunks -> [KT*P, 2*W] with the DoubleRow pairing
    row kk*128+p, col i*W+n  <->  contraction index kk*256 + 128i + p."""
    cat = np.concatenate(cols_list, axis=1)  # [D, W]
    w = cat.shape[1]
    return np.ascontiguousarray(
        cat.reshape(KT, 2, P, w).transpose(0, 2, 1, 3).reshape(KT * P, 2 * w)
    )


def _run_device(xnT, y_pred, trace=False):
    """Run the SPMD kernel; returns (T[N], R[N]) f64 and the raw results."""
    fp8np = mybir.dt.np(_FP8)
    zq = (xnT * S8).astype(np.float32).astype(fp8np)  # [D, N] fp8
    ch = lambda j: zq[:, (j % 8) * BLK:(j % 8) * BLK + BLK]
    in_maps = []
    for c in range(NCORES):
        blk = slice(c * BLK, (c + 1) * BLK)
        ypb = (
            np.ascontiguousarray(y_pred[blk])
            .reshape(MT, P, C).transpose(1, 0, 2).reshape(P, MT * C)
        )
        if c < 4:
            c4 = ch(c + 4)
            wh = ch(c)[:, 0:256]
        else:
            c4 = ch(c)
            wh = ch(c - 4)[:, 256:512]
        in_maps.append({
            "biga": _pack_cols(zq, [ch(c), ch(c + 1)]),
            "bigb": _pack_cols(zq, [ch(c + 2), ch(c + 3), c4, wh]),
            "yp": np.ascontiguousarray(ypb),
        })
    res = run_bass_kernel_spmd(
        _get_nc(), in_maps, core_ids=list(range(NCORES)), trace=trace,
    )
    T = np.zeros(N, np.float64)
    R = np.empty(N, np.float64)
    for c, r in enumerate(res.results):
        o = r["out_rs"].astype(np.float64)            # [128, 22]
        cs = r["out_cs"].astype(np.float64).reshape(4, BLK)
        for m in range(MT):
            rows = slice(c * BLK + m * P, c * BLK + (m + 1) * P)
            T[rows] += (o[:, RS_B1 + m] + o[:, RS_B2 + m]
                        + o[:, RS_B3 + m] + o[:, RS_B0 + m])
            R[rows] = o[:, RS_CE + m]
        # half-block row sums
        if c < 4:
            p0 = c * BLK
            T[p0:p0 + P] += o[:, RS_B4]
            T[p0 + P:p0 + 2 * P] += o[:, RS_B4 + 1]
        else:
            p0 = (c - 4) * BLK
            T[p0 + 2 * P:p0 + 3 * P] += o[:, RS_B4]
            T[p0 + 3 * P:p0 + 4 * P] += o[:, RS_B4 + 1]
        # column sums
        chs = lambda j: slice((j % 8) * BLK, (j % 8) * BLK + BLK)
        T[chs(c + 1)] += cs[CS_B1]
        T[chs(c + 2)] += cs[CS_B2]
        T[chs(c + 3)] += cs[CS_B3]
        T[chs(c + 4 if c < 4 else c)] += cs[CS_B4]
    return T, R, res


def kernel(layer_embeds, y_true, y_pred):
    x = np.asarray(layer_embeds, dtype=np.float32)
    yt = np.asarray(y_true).astype(np.int64)
    yp = np.asarray(y_pred, dtype=np.float32)

    # normalize rows (torch-style eps clip)
    norms = np.maximum(
        np.sqrt((x.astype(np.float64) ** 2).sum(1, keepdims=True)), 1e-8
    )
    xn = (x / norms).astype(np.float32)
    xnT = np.ascontiguousarray(xn.T)  # [D, N]

    trace = bool(int(os.environ.get("CLCE_TRACE", "0")))
    T, R, res = _run_device(xnT, yp, trace=trace)
    if trace:
        kernel.last_results = res

    # --- host-side small terms (O(N * class_size)) ---
    # P_ must match what the device summed for the same-class entries, i.e.
    # the fp8-quantized sim values, so quantize the same way here.
    fp8np = mybir.dt.np(_FP8)
    xq = (xn * S8).astype(fp8np).astype(np.float64) / S8  # device-visible xn
    counts = np.bincount(yt, minlength=C)
    P_ = np.zeros(N, np.float64)
    slot0 = np.zeros(N, np.float64)
    for cval in np.unique(yt):
        idx = np.where(yt == cval)[0]
        subq = xq[idx]
        sq = (subq @ subq.T + 1.0) * (0.5 * TAU)   # device-matching sim
        P_[idx] = np.exp(sq).sum(1)
        if len(idx) >= 2:
            # slot0 feeds the final formula directly -> use full precision
            sub = xn[idx].astype(np.float64)
            s = (sub @ sub.T + 1.0) * (0.5 * TAU)
            firstpos = np.where(np.arange(len(idx)) == 0, 1, 0)
            slot0[idx] = s[np.arange(len(idx)), firstpos]

    num_neg = N - counts[yt]
    S = T - P_
    Z = (2 * N - 2 - num_neg).astype(np.float64)
    cl = (np.log(np.exp(slot0) + S + Z) - slot0).mean()
    ce = (
        np.log(R) - yp[np.arange(N), yt].astype(np.float64)
    ).mean()
    loss = LAMBD * cl + (1.0 - LAMBD) * ce
    return np.asarray(loss, dtype=np.float32)


# revision 5
# speedup vs baseline: 1.2734x; 1.2377x over previous
"""CLCE loss kernel for Trainium2 (8 NeuronCores, SPMD) — symmetric-triangle.

Loss = 0.5 * cl + 0.5 * ce where
  cl_i = log(exp(slot0_i) + (T_i - P_i) + (2N-2 - num_neg_i)) - slot0_i
  T_i  = sum_j exp((xn_i . xn_j + 1) * 0.25)      <- O(N^2 D), on device
  P_i, slot0_i, ce assembled on host as in the validated baseline.

exp(sim) is symmetric, so only a triangle cover of the 8x8 grid of
512x512 blocks is computed (4.5 block-units per core instead of 8):
  core k: diag (k,k)  [row-sums only]
          (k, k+d) for d=1..3  [row-sums + column-sums]
          half of the d=4 block: rows chunk p=k%4 (m01 if k<4 else m23)
          x cols chunk p+4  [row-sums + column-sums]
Row sums come free from the Scalar-engine exp accumulator; column sums
are DVE adds of the bf16 exp tiles over the m-tiles followed by a
ones-weight matmul (contraction over the 128 partitions).  Host adds the
per-block RS/CS pieces into T.  Matmul work: 72 DR-fp8 matmuls of 512
cols vs 128 in the full-row version.

Pipeline: phase 1 k-chases the arriving (A[k], B2[k]) DMA pairs with 6
matmuls per arrival (b1 m0-3 + b2 m0-1, 6 psum banks) so the PE never
idles on the DMA warm-up; phase 2 runs the remaining blocks m-outer.
All bulk input DMAs ride the sync HWDGE queue back-to-back in exact
consumption order (a deep queue backlog is what keeps the DMA engines
saturated); y_pred rides the scalar queue as bf16.
"""

import os
from contextlib import ExitStack

import numpy as np

import concourse.bass as bass
import concourse.tile as tile
from concourse import bacc, mybir
from concourse.bass_utils import run_bass_kernel_spmd

N, D, C = 4096, 1024, 512
TAU = 0.5
LAMBD = 0.5
NCORES = 8
BLK = 512                  # chunk width (rows per core block)
P = 128                    # partitions
KT = D // 256              # 4 DoubleRow contraction super-tiles
MT = BLK // P              # 4 m-tiles per full block
S8 = 16.0                  # fp8 pre-scale for the embeddings
AW = 2 * BLK               # A region: c0 (W + diag X) | c1
RW = 2 * BLK + 256         # BR region: c3 | c4 | wh

_F32 = mybir.dt.float32
_BF16 = mybir.dt.bfloat16
_FP8 = mybir.dt.float8e4
_EXP = mybir.ActivationFunctionType.Exp
_DR = mybir.MatmulPerfMode.DoubleRow

# OUTSB column map
RS_B1, RS_B2, RS_B3, RS_B4, RS_B0, RS_CE = 0, 4, 8, 12, 14, 18
NRS = 22
# OUTCS slot map (512 cols each)
CS_B1, CS_B2, CS_B3, CS_B4 = 0, 1, 2, 3


def _build_kernel(tc, biga, bigb2, bigbr, yp, out_rs, out_cs):
    """Emit the per-core Tile kernel.

    biga:  [KT*P, 2*AW]  fp8  per k: [c0 | c1] (W + diag X + b1 X)
    bigb2: [KT*P, 2*BLK] fp8  per k: c2 (b2 X)
    bigbr: [KT*P, 2*RW]  fp8  per k: [c3 | c4 | wh]
    yp:    [P, MT*C]     bf16 this core's y_pred block, partition-major
    out_rs: [P, 22]      f32  18 sim row-sum cols + 4 CE row-sums
    out_cs: [1, 2048]    f32  4 x 512 column-sum vectors
    """
    nc = tc.nc
    act_scale = 0.5 * TAU / (S8 * S8)
    with ExitStack() as ctx:
        pers = ctx.enter_context(tc.tile_pool(name="pers", bufs=1))
        epool = ctx.enter_context(tc.tile_pool(name="epool", bufs=3))
        cepool = ctx.enter_context(tc.tile_pool(name="cepool", bufs=2))
        apool = ctx.enter_context(tc.tile_pool(name="apool", bufs=4))
        psum = ctx.enter_context(
            tc.tile_pool(name="psum", bufs=6, space=bass.MemorySpace.PSUM)
        )
        cpsum = ctx.enter_context(
            tc.tile_pool(name="cpsum", bufs=2, space=bass.MemorySpace.PSUM)
        )

        A = [pers.tile([P, 2, AW], _FP8, name=f"ba{k}", tag=f"ba{k}")
             for k in range(KT)]
        B2 = [pers.tile([P, 2, BLK], _FP8, name=f"b2{k}", tag=f"b2{k}")
              for k in range(KT)]
        BR = [pers.tile([P, 2, RW], _FP8, name=f"br{k}", tag=f"br{k}")
              for k in range(KT)]
        YPB = pers.tile([P, MT * C], _BF16)
        OUTSB = pers.tile([P, NRS], _F32)
        OUTCS = pers.tile([1, 4 * BLK], _F32)
        bias_s = pers.tile([P, 1], _F32)
        bias_z = pers.tile([P, 1], _F32)
        warm = pers.tile([P, 1], _F32)
        ZW = pers.tile([P, 512], _BF16)
        ONES = pers.tile([P, P], _BF16)

        # input DMAs first: sync HWDGE queue, exact consumption order
        a3 = biga.rearrange("r (i n) -> r i n", i=2)
        b23 = bigb2.rearrange("r (i n) -> r i n", i=2)
        br3 = bigbr.rearrange("r (i n) -> r i n", i=2)
        for k in range(KT):
            nc.sync.dma_start(A[k][:], a3[k * P:(k + 1) * P, :, :])
            nc.sync.dma_start(B2[k][:], b23[k * P:(k + 1) * P, :, :])
        for k in range(KT):
            nc.sync.dma_start(BR[k][:], br3[k * P:(k + 1) * P, :, :])

        # vector engine: warm-up operands (its preamble ends earliest and
        # it has no other early work)
        nc.vector.memset(ZW[:], 0.0)
        nc.vector.memset(ONES[:], 1.0)
        # gpsimd: activation biases
        nc.gpsimd.memset(bias_z[:], 0.0)
        nc.gpsimd.memset(bias_s[:], 0.5 * TAU)

        # scalar: y_pred DMA rides the scalar HWDGE queue; exp-table warm
        nc.scalar.dma_start(YPB[:], yp[:])
        nc.scalar.activation(warm[:], bias_z[:], _EXP, bias=bias_z[:], scale=1.0)

        # PE warm-up: dummy matmuls flip the HAM clock gate while the
        # first input DMAs are in flight
        wps = psum.tile([P, BLK], _F32, tag="ps")
        for _ in range(3):
            nc.tensor.matmul(wps[:], ZW[:, 0:P], ZW[:], start=True, stop=True)

        def act_rs(ps, col):
            """exp + row-sum accumulate; returns the bf16 exp tile."""
            et = epool.tile([P, BLK], _BF16, tag="et")
            nc.scalar.activation(
                et[:], ps[:], _EXP, bias=bias_s[:], scale=act_scale,
                accum_out=OUTSB[:, col:col + 1],
            )
            return et

        def ones_mm(acc, slot):
            cps = cpsum.tile([P, BLK], _F32, tag="cps")
            nc.tensor.matmul(cps[:], ONES[:], acc[:], start=True, stop=True)
            nc.vector.tensor_copy(
                OUTCS[0:1, slot * BLK:(slot + 1) * BLK], cps[0:1, :]
            )

        def cs_adds(ets, n):
            """pairwise-tree DVE adds of the exp tiles -> one bf16 tile."""
            if n == 2:
                x = apool.tile([P, BLK], _BF16, tag="acc")
                nc.vector.tensor_add(x[:], ets[0][:], ets[1][:])
                return x
            x01 = apool.tile([P, BLK], _BF16, tag="acc")
            nc.vector.tensor_add(x01[:], ets[0][:], ets[1][:])
            x23 = apool.tile([P, BLK], _BF16, tag="acc")
            nc.vector.tensor_add(x23[:], ets[2][:], ets[3][:])
            xs = apool.tile([P, BLK], _BF16, tag="acc")
            nc.vector.tensor_add(xs[:], x01[:], x23[:])
            return xs

        def mm(ps, k, m, xsrc, xoff, wsrc=None, woff=0):
            nc.tensor.matmul(
                ps[:],
                (wsrc or A)[k][:, :, woff + m * P: woff + (m + 1) * P],
                xsrc[k][:, :, xoff: xoff + BLK],
                start=(k == 0),
                stop=(k == KT - 1),
                perf_mode=_DR,
            )

        # --- phase 1: k-chase the (A[k], B2[k]) arrivals ---
        ps1 = [psum.tile([P, BLK], _F32, tag="ps", name=f"ps1_{m}")
               for m in range(MT)]
        ps2 = [psum.tile([P, BLK], _F32, tag="ps", name=f"ps2_{m}")
               for m in range(2)]
        for k in range(KT):
            for m in range(MT):
                mm(ps1[m], k, m, A, BLK)
            for m in range(2):
                mm(ps2[m], k, m, B2, 0)
            if k < KT - 1:  # HAM keep-alive if the next pair is late
                nc.tensor.matmul(wps[:], ZW[:, 0:P], ZW[:], start=True, stop=True)

        ets1 = [act_rs(ps1[m], RS_B1 + m) for m in range(MT)]
        csa1 = cs_adds(ets1, 4)

        # CE while ScalarE has slack: R[p, t] = sum_c exp(y_pred)
        for t in range(MT):
            etc = cepool.tile([P, C], _F32, tag="etce")
            nc.scalar.activation(
                etc[:], YPB[:, t * C:(t + 1) * C], _EXP,
                bias=bias_z[:], scale=1.0,
                accum_out=OUTSB[:, RS_CE + t:RS_CE + t + 1],
            )

        # --- phase 2 ---
        # b2 m2/m3 (m-outer), ACTs for b2 m0..3
        ets2 = [None] * MT
        psb = []
        for m in (2, 3):
            ps = psum.tile([P, BLK], _F32, tag="ps")
            for k in range(KT):
                mm(ps, k, m, B2, 0)
            psb.append(ps)
        ets2[0] = act_rs(ps2[0], RS_B2 + 0)
        ets2[1] = act_rs(ps2[1], RS_B2 + 1)
        ets2[2] = act_rs(psb[0], RS_B2 + 2)
        ets2[3] = act_rs(psb[1], RS_B2 + 3)
        csa2 = cs_adds(ets2, 4)

        # b3 (X = c3 at BR offset 0); onesMM(b1) after m0, (b2) after m2
        ets3 = []
        for m in range(MT):
            ps = psum.tile([P, BLK], _F32, tag="ps")
            for k in range(KT):
                mm(ps, k, m, BR, 0)
            ets3.append(act_rs(ps, RS_B3 + m))
            if m == 0:
                ones_mm(csa1, CS_B1)
            if m == 2:
                ones_mm(csa2, CS_B2)
        csa3 = cs_adds(ets3, 4)

        # b4 (half: W = wh at BR 2*BLK, X = c4 at BR BLK)
        ets4 = []
        for m in range(2):
            ps = psum.tile([P, BLK], _F32, tag="ps")
            for k in range(KT):
                mm(ps, k, m, BR, BLK, wsrc=BR, woff=2 * BLK)
            ets4.append(act_rs(ps, RS_B4 + m))
        csa4 = cs_adds(ets4, 2)

        # b0 (diag, X = c0 at A offset 0): RS only, short tail
        for m in range(MT):
            ps = psum.tile([P, BLK], _F32, tag="ps")
            for k in range(KT):
                mm(ps, k, m, A, 0)
            act_rs(ps, RS_B0 + m)
            if m == 0:
                ones_mm(csa3, CS_B3)
            if m == 1:
                ones_mm(csa4, CS_B4)
            if m == 2:
                nc.gpsimd.dma_start(out_cs[:], OUTCS[:])

        nc.scalar.dma_start(out_rs[:], OUTSB[:])


_NC_CACHE = None


def _get_nc():
    global _NC_CACHE
    if _NC_CACHE is None:
        nc = bacc.Bacc(
            "TRN2", target_bir_lowering=False, debug=False,
            enable_asserts=False, num_devices=NCORES,
        )
        biga_d = nc.dram_tensor("biga", [KT * P, 2 * AW], _FP8, kind="ExternalInput")
        bigb2_d = nc.dram_tensor("bigb2", [KT * P, 2 * BLK], _FP8, kind="ExternalInput")
        bigbr_d = nc.dram_tensor("bigbr", [KT * P, 2 * RW], _FP8, kind="ExternalInput")
        yp_d = nc.dram_tensor("yp", [P, MT * C], _BF16, kind="ExternalInput")
        out_rs_d = nc.dram_tensor("out_rs", [P, NRS], _F32, kind="ExternalOutput")
        out_cs_d = nc.dram_tensor("out_cs", [1, 4 * BLK], _F32, kind="ExternalOutput")
        with tile.TileContext(nc) as tc:
            _build_kernel(
                tc, biga_d.ap(), bigb2_d.ap(), bigbr_d.ap(), yp_d.ap(),
                out_rs_d.ap(), out_cs_d.ap(),
            )
        nc.compile()
        _NC_CACHE = nc
    return _NC_CACHE


def _pack_cols(zq, cols_list):
    """[D, *] fp8 col-chunks -> [KT*P, 2*W] with the DoubleRow pairing
    row kk*128+p, col i*W+n  <->  contraction index kk*256 + 128i + p."""
    cat = np.concatenate(cols_list, axis=1)  # [D, W]
    w = cat.shape[1]
    return np.ascontiguousarray(
        cat.reshape(KT, 2, P, w).transpose(0, 2, 1, 3).reshape(KT * P, 2 * w)
    )


def _run_device(xnT, y_pred, trace=False):
    """Run the SPMD kernel; returns (T[N], R[N]) f64 and the raw results."""
    fp8np = mybir.dt.np(_FP8)
    bf16np = mybir.dt.np(_BF16)
    zq = (xnT * S8).astype(np.float32).astype(fp8np)  # [D, N] fp8
    ch = lambda j: zq[:, (j % 8) * BLK:(j % 8) * BLK + BLK]
    in_maps = []
    for c in range(NCORES):
        blk = slice(c * BLK, (c + 1) * BLK)
        ypb = (
            np.ascontiguousarray(y_pred[blk])
            .reshape(MT, P, C).transpose(1, 0, 2).reshape(P, MT * C)
        )
        if c < 4:
            c4 = ch(c + 4)
            wh = ch(c)[:, 0:256]
        else:
            c4 = ch(c)
            wh = ch(c - 4)[:, 256:512]
        in_maps.append({
            "biga": _pack_cols(zq, [ch(c), ch(c + 1)]),
            "bigb2": _pack_cols(zq, [ch(c + 2)]),
            "bigbr": _pack_cols(zq, [ch(c + 3), c4, wh]),
            "yp": np.ascontiguousarray(ypb).astype(bf16np),
        })
    res = run_bass_kernel_spmd(
        _get_nc(), in_maps, core_ids=list(range(NCORES)), trace=trace,
    )
    T = np.zeros(N, np.float64)
    R = np.empty(N, np.float64)
    for c, r in enumerate(res.results):
        o = r["out_rs"].astype(np.float64)            # [128, 22]
        cs = r["out_cs"].astype(np.float64).reshape(4, BLK)
        for m in range(MT):
            rows = slice(c * BLK + m * P, c * BLK + (m + 1) * P)
            T[rows] += (o[:, RS_B1 + m] + o[:, RS_B2 + m]
                        + o[:, RS_B3 + m] + o[:, RS_B0 + m])
            R[rows] = o[:, RS_CE + m]
        # half-block row sums
        if c < 4:
            p0 = c * BLK
            T[p0:p0 + P] += o[:, RS_B4]
            T[p0 + P:p0 + 2 * P] += o[:, RS_B4 + 1]
        else:
            p0 = (c - 4) * BLK
            T[p0 + 2 * P:p0 + 3 * P] += o[:, RS_B4]
            T[p0 + 3 * P:p0 + 4 * P] += o[:, RS_B4 + 1]
        # column sums
        chs = lambda j: slice((j % 8) * BLK, (j % 8) * BLK + BLK)
        T[chs(c + 1)] += cs[CS_B1]
        T[chs(c + 2)] += cs[CS_B2]
        T[chs(c + 3)] += cs[CS_B3]
        T[chs(c + 4 if c < 4 else c)] += cs[CS_B4]
    return T, R, res


def kernel(layer_embeds, y_true, y_pred):
    x = np.asarray(layer_embeds, dtype=np.float32)
    yt = np.asarray(y_true).astype(np.int64)
    yp = np.asarray(y_pred, dtype=np.float32)

    # normalize rows (torch-style eps clip)
    norms = np.maximum(
        np.sqrt((x.astype(np.float64) ** 2).sum(1, keepdims=True)), 1e-8
    )
    xn = (x / norms).astype(np.float32)
    xnT = np.ascontiguousarray(xn.T)  # [D, N]

    trace = bool(int(os.environ.get("CLCE_TRACE", "0")))
    T, R, res = _run_device(xnT, yp, trace=trace)
    if trace:
        kernel.last_results = res

    # --- host-side small terms (O(N * class_size)) ---
    # P_ must match what the device summed for the same-class entries, i.e.
    # the fp8-quantized sim values, so quantize the same way here.
    fp8np = mybir.dt.np(_FP8)
    xq = (xn * S8).astype(fp8np).astype(np.float64) / S8  # device-visible xn
    counts = np.bincount(yt, minlength=C)
    P_ = np.zeros(N, np.float64)
    slot0 = np.zeros(N, np.float64)
    for cval in np.unique(yt):
        idx = np.where(yt == cval)[0]
        subq = xq[idx]
        sq = (subq @ subq.T + 1.0) * (0.5 * TAU)   # device-matching sim
        P_[idx] = np.exp(sq).sum(1)
        if len(idx) >= 2:
            # slot0 feeds the final formula directly -> use full precision
            sub = xn[idx].astype(np.float64)
            s = (sub @ sub.T + 1.0) * (0.5 * TAU)
            firstpos = np.where(np.arange(len(idx)) == 0, 1, 0)
            slot0[idx] = s[np.arange(len(idx)), firstpos]

    num_neg = N - counts[yt]
    S = T - P_
    Z = (2 * N - 2 - num_neg).astype(np.float64)
    cl = (np.log(np.exp(slot0) + S + Z) - slot0).mean()
    ce = (
        np.log(R) - yp[np.arange(N), yt].astype(np.float64)
    ).mean()
    loss = LAMBD * cl + (1.0 - LAMBD) * ce
    return np.asarray(loss, dtype=np.float32)
